# revision 3
# baseline (speedup 1.0000x reference)
"""Bahdanau attention TRN2 kernel.

Problem shapes (hardcoded):
    encoder_outputs: [64, 2048, 512] f32
    decoder_hidden:  [64, 512] f32
    W1, W2: [512, 512]; b1, b2: [512]; V: [512]; bv: [1]
Returns (context [64, 512] f32, attn [64, 2048] f32), matching the reference.

Sharding: data-parallel over batch across 8 NeuronCores (8 batches/core).

Per-core algorithm (per batch b):
    scores = tanh(enc @ W1.T + qb) @ V        (qb = b1 + b2 + dec @ W2.T, host-computed;
                                               bv dropped - softmax is shift-invariant)
    attn   = softmax(scores)                  (no max-subtract: |scores| <= sum|V| ~ 12)
    context= attn @ enc

Device layouts:
    enc_t  [h, s] bf16  - transposed enc (host prep), moving operand of the projection matmul
    enc_g  [p, j*512+h] bf16 - enc rows grouped s = p*16 + j, rhs of the context matmul
    scores are produced as PSUM [1, 512] rows at partitions {0,32,64,96} (PE col-tiling),
    reshaped to [128, 16] (s = p*16 + j) via small SBUF->SBUF DMAs.
"""

import numpy as np
import ml_dtypes

import concourse.bass as bass
import concourse.tile as tile
from concourse import bacc, mybir
from concourse._compat import with_exitstack

BF16 = mybir.dt.bfloat16
F32 = mybir.dt.float32
NPBF16 = ml_dtypes.bfloat16

B, S, H = 64, 2048, 512
NCORES = 8
BPC = B // NCORES          # batches per core
ST = 4                     # "supertiles" per batch: 512 s-rows each (vdot col-tile groups)
NC_CHUNK = H // 128        # 4 chunks of 128 along h / o
NJ = S // 128              # 16 s-groups of 128 for the context matmul

AFT = mybir.ActivationFunctionType


@with_exitstack
def _bahdanau_tile_kernel(ctx, tc, enc_g, enc_t, w1t, vvec, qbt, attn_out, ctx_out):
    nc = tc.nc

    const = ctx.enter_context(tc.tile_pool(name="const", bufs=1))
    encn_pool = ctx.enter_context(tc.tile_pool(name="encn", bufs=3))
    enct_pool = ctx.enter_context(tc.tile_pool(name="enct", bufs=8))
    energy_pool = ctx.enter_context(tc.tile_pool(name="energy", bufs=20))
    sm_pool = ctx.enter_context(tc.tile_pool(name="sm", bufs=2))
    pe_psum = ctx.enter_context(tc.tile_pool(name="pe_ps", bufs=3, space="PSUM"))
    sc_psum = ctx.enter_context(tc.tile_pool(name="sc_ps", bufs=2, space="PSUM"))
    small_psum = ctx.enter_context(tc.tile_pool(name="small_ps", bufs=3, space="PSUM"))

    # --- constants ---
    w1t_sb = []   # [128, 512] bf16 per h-chunk, cols = o
    vv_sb = []    # [128, 1] bf16 per o-chunk
    qbt_sb = []   # [128, BPC] f32 per o-chunk
    for c in range(NC_CHUNK):
        wt = const.tile([128, H], BF16, name=f"w1t{c}", tag=f"w1t{c}")
        nc.sync.dma_start(wt[:], w1t[c * 128:(c + 1) * 128, :])
        w1t_sb.append(wt)
        vv = const.tile([128, 1], BF16, name=f"vv{c}", tag=f"vv{c}")
        nc.sync.dma_start(vv[:], vvec[c * 128:(c + 1) * 128, :])
        vv_sb.append(vv)
        qb = const.tile([128, BPC], F32, name=f"qbt{c}", tag=f"qbt{c}")
        nc.sync.dma_start(qb[:], qbt[c * 128:(c + 1) * 128, :])
        qbt_sb.append(qb)
    ones_col = const.tile([128, 1], F32, name="ones_col", tag="ones_col")
    nc.vector.memset(ones_col[:], 1.0)
    ones_row = const.tile([1, 128], F32, name="ones_row", tag="ones_row")
    nc.vector.memset(ones_row[:], 1.0)

    for b in range(BPC):
        # --- loads ---
        encn = encn_pool.tile([128, NJ * H], BF16, name="encn", tag="encn")
        nc.sync.dma_start(encn[:], enc_g[b * 128:(b + 1) * 128, :])
        enct = []
        for c in range(NC_CHUNK):
            et = enct_pool.tile([128, S], BF16, name="enct", tag="enct")
            nc.sync.dma_start(et[:], enc_t[b * H + c * 128:b * H + (c + 1) * 128, :])
            enct.append(et)

        # --- projection matmul + tanh (transposed layout: partitions = o) ---
        energies = {}
        for st in range(ST):
            for oc in range(NC_CHUNK):
                pe = pe_psum.tile([128, 512], F32, name="pe", tag="pe")
                for hc in range(NC_CHUNK):
                    nc.tensor.matmul(
                        pe[:],
                        lhsT=w1t_sb[hc][:, oc * 128:(oc + 1) * 128],
                        rhs=enct[hc][:, st * 512:(st + 1) * 512],
                        start=(hc == 0),
                        stop=(hc == NC_CHUNK - 1),
                    )
                en = energy_pool.tile([128, 512], BF16, name="en", tag="en")
                nc.scalar.activation(en[:], pe[:], AFT.Tanh, bias=qbt_sb[oc][:, b:b + 1])
                energies[(st, oc)] = en

        # --- V-dot: scores[st*512 + n] at psum partition 32*st (col-tiled) ---
        sc = sc_psum.tile([128, 512], F32, name="sc", tag="sc")
        for oc in range(NC_CHUNK):
            for st in range(ST):
                nc.tensor.matmul(
                    sc[32 * st:32 * st + 1, :],
                    lhsT=vv_sb[oc][:],
                    rhs=energies[(st, oc)][:],
                    start=(oc == 0),
                    stop=(oc == NC_CHUNK - 1),
                    tile_position=(0, 32 * st),
                )

        # --- drain scores to SBUF, reshape to [128, 16] (s = p*16 + j) ---
        sc4 = sm_pool.tile([128, 512], F32, name="sc4", tag="sc4")
        for st in range(ST):
            nc.vector.tensor_copy(sc4[32 * st:32 * st + 1, :], sc[32 * st:32 * st + 1, :])
        scT = sm_pool.tile([128, NJ], F32, name="scT", tag="scT")
        for st in range(ST):
            nc.sync.dma_start(scT[st * 32:(st + 1) * 32, :], sc4[32 * st:32 * st + 1, :])

        # --- softmax (unnormalized exp; fold 1/sum in at the end) ---
        expT = sm_pool.tile([128, NJ], F32, name="expT", tag="expT")
        expP = sm_pool.tile([128, 1], F32, name="expP", tag="expP")
        nc.scalar.activation(expT[:], scT[:], AFT.Exp, accum_out=expP[:])
        expTb = sm_pool.tile([128, NJ], BF16, name="expTb", tag="expTb")
        nc.vector.tensor_copy(expTb[:], expT[:])

        ps_sum = small_psum.tile([1, 1], F32, name="ps_sum", tag="small")
        nc.tensor.matmul(ps_sum[:], lhsT=ones_col[:], rhs=expP[:], start=True, stop=True)
        recip = sm_pool.tile([1, 1], F32, name="recip", tag="recip")
        nc.vector.reciprocal(recip[:], ps_sum[:])

        # --- context matmul (unnormalized), then scale by 1/sum ---
        ps_ctx = small_psum.tile([1, 512], F32, name="ps_ctx", tag="small")
        for j in range(NJ):
            nc.tensor.matmul(
                ps_ctx[:],
                lhsT=expTb[:, j:j + 1],
                rhs=encn[:, j * 512:(j + 1) * 512],
                start=(j == 0),
                stop=(j == NJ - 1),
            )
        ctx_sb = sm_pool.tile([1, 512], F32, name="ctx_sb", tag="ctx_sb")
        nc.scalar.activation(ctx_sb[:], ps_ctx[:], AFT.Copy, scale=recip[:])
        nc.sync.dma_start(ctx_out[b:b + 1, :], ctx_sb[:])

        # --- attn = expT / sum ---
        ps_bc = small_psum.tile([128, 1], F32, name="ps_bc", tag="small")
        nc.tensor.matmul(ps_bc[:], lhsT=ones_row[:], rhs=recip[:], start=True, stop=True)
        recip_bc = sm_pool.tile([128, 1], F32, name="recip_bc", tag="recip_bc")
        nc.vector.tensor_copy(recip_bc[:], ps_bc[:])
        attn_sb = sm_pool.tile([128, NJ], F32, name="attn_sb", tag="attn_sb")
        nc.vector.tensor_scalar_mul(attn_sb[:], expT[:], recip_bc[:])
        nc.sync.dma_start(attn_out[b * 128:(b + 1) * 128, :], attn_sb[:])


def build_program():
    nc = bacc.Bacc(
        "TRN2",
        target_bir_lowering=False,
        debug=False,
        enable_asserts=False,
        num_devices=NCORES,
    )
    enc_g = nc.dram_tensor("enc_g", [BPC * 128, NJ * H], BF16, kind="ExternalInput").ap()
    enc_t = nc.dram_tensor("enc_t", [BPC * H, S], BF16, kind="ExternalInput").ap()
    w1t = nc.dram_tensor("w1t", [H, H], BF16, kind="ExternalInput").ap()
    vvec = nc.dram_tensor("vvec", [H, 1], BF16, kind="ExternalInput").ap()
    qbt = nc.dram_tensor("qbt", [H, BPC], F32, kind="ExternalInput").ap()
    attn_out = nc.dram_tensor("attn_out", [BPC * 128, NJ], F32, kind="ExternalOutput").ap()
    ctx_out = nc.dram_tensor("ctx_out", [BPC, H], F32, kind="ExternalOutput").ap()

    with tile.TileContext(nc) as tc:
        _bahdanau_tile_kernel(tc, enc_g, enc_t, w1t, vvec, qbt, attn_out, ctx_out)
    nc.compile()
    return nc


def prepare_in_maps(encoder_outputs, decoder_hidden, W1, b1, W2, b2, V, bv):
    enc = np.asarray(encoder_outputs, dtype=np.float32)
    dec = np.asarray(decoder_hidden, dtype=np.float32)
    W1 = np.asarray(W1, dtype=np.float32)
    W2 = np.asarray(W2, dtype=np.float32)
    b1 = np.asarray(b1, dtype=np.float32)
    b2 = np.asarray(b2, dtype=np.float32)
    V = np.asarray(V, dtype=np.float32)

    enc_bf = enc.astype(NPBF16)
    # s = p*16 + j grouping: [B, 128, 16, H] -> per core [BPC*128, 16*H]
    enc_g_all = enc_bf.reshape(B, 128, NJ, H)
    enc_t_all = np.ascontiguousarray(enc_bf.transpose(0, 2, 1))  # [B, H, S]

    qb_all = (b1 + b2 + dec @ W2.T).astype(np.float32)  # [B, H]
    w1t_np = np.ascontiguousarray(W1.T).astype(NPBF16)  # [h, o]
    vvec_np = V.astype(NPBF16).reshape(H, 1)

    in_maps = []
    for c in range(NCORES):
        bs = slice(c * BPC, (c + 1) * BPC)
        in_maps.append({
            "enc_g": np.ascontiguousarray(enc_g_all[bs]).reshape(BPC * 128, NJ * H),
            "enc_t": np.ascontiguousarray(enc_t_all[bs]).reshape(BPC * H, S),
            "w1t": w1t_np,
            "vvec": vvec_np,
            "qbt": np.ascontiguousarray(qb_all[bs].T),
        })
    return in_maps


_CACHED_NC = None


def kernel(encoder_outputs, decoder_hidden, W1, b1, W2, b2, V, bv, _trace=False):
    global _CACHED_NC
    from concourse import bass_utils

    if _CACHED_NC is None:
        _CACHED_NC = build_program()
    nc = _CACHED_NC

    in_maps = prepare_in_maps(encoder_outputs, decoder_hidden, W1, b1, W2, b2, V, bv)
    res = bass_utils.run_bass_kernel_spmd(
        nc, in_maps, core_ids=list(range(NCORES)), trace=_trace,
    )
    context = np.concatenate(
        [res.results[c]["ctx_out"] for c in range(NCORES)], axis=0
    ).astype(np.float32)
    attn = np.concatenate(
        [res.results[c]["attn_out"].reshape(BPC, S) for c in range(NCORES)], axis=0
    ).astype(np.float32)
    if _trace:
        kernel._last_results = res
    return context, attn
